# revision 38
# baseline (speedup 1.0000x reference)
"""Binarized 3x3 conv (stride 1, pad 1) + bias on 8 Trainium2 NeuronCores.

Full problem: x[32,256,56,56] f32, weight[256,256,3,3] f32, bias[256] f32
-> y[32,256,56,56] f32 with y = conv2d(sign(x), sign(weight), pad=1) + bias
(sign(t) = +1 for t >= 0 else -1).

Sharding: data-parallel over batch. Each of the 8 cores gets 4 images and a
replicated copy of the weights; the host concatenates the 8 output shards.

Per-core kernel (v10): 1-D Winograd F(2,3) along H, host-side input transform.
  The 3 kh-taps collapse into 4 m-plane matmul groups over stride-2 row
  tiles, cutting streamed PE columns 1.5x vs direct conv (12 matmuls of
  N=406 per 7-tile-row chunk instead of 18 equivalent direct columns).
  Everything stays exact: the host binarizes x to +/-1 and ships the 4
  RT input-transform planes (d_a +/- d_b in {-2,0,2}) as fp8; the weight
  transform G@g has entries in {+/-0.5, +/-1, +/-1.5} (exact in e4m3);
  products are multiples of 0.5 accumulated in f32 PSUM; y rows are
  m0+m1+m2 / m1-m2-m3 -- half-integer sums well under 2048, so the fp16
  output rounds only the +bias term (~1e-4 rel err).

  - per (image, co_blk, chunk of 7 tile rows): 12 DoubleRow fp8 matmuls
    (4 m-planes x 3 kw-taps) into 4 PSUM banks (~2.03us/chunk, the pace);
  - drain/inverse-transform per chunk, split to fit engine budgets
    (PSUM readable only by Scalar/DVE, one operand per op):
      Scalar: s1 = m1 + bias, s3 = m3            (2 ACTs, 1.17us)
      DVE:    t = m0 + s1; ye = t + m2; u = s1 - m2   (3 ops, 1.66us)
      GpSimd: yo = u - s3                        (1 op, 1.19us)
  - y ships fp16 (halved writeback), host upcasts to f32.
"""

import ml_dtypes
import numpy as np

import concourse.bacc as bacc
import concourse.mybir as mybir
import concourse.tile as tile
from concourse.bass_utils import run_bass_kernel_spmd

F32 = mybir.dt.float32
FP16 = mybir.dt.float16
BF16 = mybir.dt.bfloat16
FP8 = mybir.dt.float8e4
AF = mybir.ActivationFunctionType
ALU = mybir.AluOpType
DR = mybir.MatmulPerfMode.DoubleRow

N_CORES = 8
H = W = 56
WP = 58            # padded row width
CIN = 256
COUT = 256
CI_BLKS = 2
CO_BLKS = 2
NRT = 28           # F(2,3) tile rows (2 output rows each)
RQ = 7             # tile rows per chunk
NQ = NRT // RQ     # 4 chunks per (image, co_blk)
NV = RQ * WP       # 406 matmul moving free size
RTL = 1632         # per-(ci_blk, m) RT plane elems (2 guard + 28*58 + pad)
JUNK_MM = 4


def _build_conv(tc, y_ap, rt_ap, wt_ap, b_ap, n_imgs):
    nc = tc.nc

    with (
        tc.tile_pool(name="consts", bufs=1) as consts,
        tc.tile_pool(name="lhst", bufs=1) as lhst_pool,
        tc.tile_pool(name="rt", bufs=1) as rt_pool,
        tc.tile_pool(name="s1", bufs=3) as s1_pool,
        tc.tile_pool(name="s3", bufs=3) as s3_pool,
        tc.tile_pool(name="tmpe", bufs=2) as tmpe_pool,
        tc.tile_pool(name="tmpo", bufs=2) as tmpo_pool,
        tc.tile_pool(name="outsb", bufs=4) as out_pool,
        tc.tile_pool(name="psum", bufs=8, space="PSUM") as psum_pool,
    ):
        junk = consts.tile([128, 512], BF16, name="junk")
        nc.gpsimd.memset(junk, 0.0)

        # Winograd weights: [128ci_p, 4m, 2ci_blk, 3kw, 256co] fp8, i-major
        # so the m={0,1} pair (needed by the first chunk) lands first.
        # Flattened partition-major transfers: the DMA packetizes at the
        # innermost AP dim, and sub-KB packets crawl (~38B/ns observed).
        lhst = lhst_pool.tile([128, 4, CI_BLKS, 3, COUT], FP8)
        for i in (0, 2):
            nc.scalar.dma_start(
                out=lhst[:, i:i + 2].rearrange("p i a k o -> p (i a k o)"),
                in_=wt_ap[:, i:i + 2].rearrange("p i a k o -> p (i a k o)"))
        bias_sb = consts.tile([128, CO_BLKS], F32)
        nc.scalar.dma_start(out=bias_sb,
                            in_=b_ap.rearrange("(b p) -> p b", p=128))

        # one rt buffer per image: any rotation adds write-after-read deps
        # that hold late transfers hostage to old readers
        rts = [rt_pool.tile([128, CI_BLKS, 4, RTL], FP8, name=f"rt{j}")
               for j in range(n_imgs)]

        def dma_rt_whole(n, eng):
            for cb in range(CI_BLKS):
                # flatten (m, f): one 6528B run per partition
                eng.dma_start(
                    out=rts[n][:, cb].rearrange("p i f -> p (i f)"),
                    in_=rt_ap[n, cb].rearrange("p i f -> p (i f)"))

        def dma_rt_piece(n, o0, o1):
            for cb in range(CI_BLKS):
                nc.sync.dma_start(out=rts[n][:, cb, :, o0:o1],
                                  in_=rt_ap[n, cb][:, :, o0:o1])

        def junk_mm():
            jps = psum_pool.tile([128, 512], F32, name="ps", tag="ps")
            nc.tensor.matmul(jps, junk[:, :128], junk, start=True, stop=True)

        def chunk(n, c, q, rtt, osb, per_chunk_y, tail=False):
            pms = [psum_pool.tile([128, NV], F32, name="ps", tag="ps")
                   for _ in range(4)]
            for i in range(4):
                for kw in range(3):
                    base = WP * RQ * q + kw
                    nc.tensor.matmul(
                        pms[i],
                        lhst[:, i, 0:2, kw, c * 128:(c + 1) * 128],
                        rtt[:, 0:2, i, base:base + NV],
                        start=(kw == 0), stop=(kw == 2), perf_mode=DR)
            pmv = [p.rearrange("p (r w) -> p r w", w=WP)[:, :, 1:57]
                   for p in pms]
            yv = osb.rearrange("p (h2 t w) -> p h2 t w", t=2, w=W)
            ye = yv[:, RQ * q:RQ * (q + 1), 0, :]
            yo = yv[:, RQ * q:RQ * (q + 1), 1, :]
            # PSUM access rules: GpSimd none, DVE/Scalar one operand per op.
            s1 = s1_pool.tile([128, RQ, W], F32, name="s1", tag="s1")
            nc.scalar.activation(out=s1, in_=pmv[1], func=AF.Identity,
                                 bias=bias_sb[:, c:c + 1], scale=1.0)
            s3 = s3_pool.tile([128, RQ, W], F32, name="s3", tag="s3")
            nc.scalar.activation(out=s3, in_=pmv[3], func=AF.Identity)
            tmp_e = tmpe_pool.tile([128, RQ, W], F32, name="te", tag="te")
            nc.vector.tensor_tensor(tmp_e, pmv[0], s1, ALU.add)
            nc.vector.tensor_tensor(ye, tmp_e, pmv[2], ALU.add)
            tmp_o = tmpo_pool.tile([128, RQ, W], F32, name="to", tag="to")
            nc.vector.tensor_tensor(tmp_o, s1, pmv[2], ALU.subtract)
            # final chunk: keep the drain tail off slow GpSimd
            (nc.vector if tail else nc.gpsimd).tensor_tensor(
                yo, tmp_o, s3, ALU.subtract)
            if per_chunk_y:
                dma_y(n, c, osb, 2 * RQ * W * q, 2 * RQ * W * (q + 1))
            elif q == 1:
                dma_y(n, c, osb, 0, H * W // 2)

        def dma_y(n, c, osb, lo, hi):
            nc.scalar.dma_start(
                out=y_ap[n, c * 128:(c + 1) * 128, lo:hi],
                in_=osb[:, lo:hi])

        def coblk(n, c, rtt, per_chunk_y=False):
            osb = out_pool.tile([128, H * W], FP16, name="osb", tag="osb")
            for q in range(NQ):
                chunk(n, c, q, rtt, osb, per_chunk_y,
                      tail=per_chunk_y and q == NQ - 1)
            if not per_chunk_y:
                dma_y(n, c, osb, H * W // 2, H * W)

        # --- input schedule: images 1 and 3 stream whole on the gpsimd
        # queue starting immediately; image 0 arrives in chunk-aligned
        # pieces on sync (interleaved with the ramp chunks so the dep
        # tracker can't chain early chunks onto late pieces), image 2
        # follows on sync. Pieces overlap 2 cols (chunk q streams 2 cols
        # into row 7(q+1)'s guard-zero region).
        if n_imgs > 1:
            dma_rt_whole(1, nc.gpsimd)
        if n_imgs > 3:
            dma_rt_whole(3, nc.gpsimd)
        dma_rt_piece(0, 0, WP * RQ + 2)
        for _ in range(JUNK_MM):
            junk_mm()
        osb0 = out_pool.tile([128, H * W], FP16, name="osb", tag="osb")
        for q in range(NQ):
            chunk(0, 0, q, rts[0], osb0, False)
            if q < NQ - 1:
                dma_rt_piece(0, WP * RQ * (q + 1),
                             min(WP * RQ * (q + 2) + 2, RTL))
        dma_y(0, 0, osb0, H * W // 2, H * W)
        if n_imgs > 2:
            dma_rt_whole(2, nc.sync)

        for n in range(n_imgs):
            if n > 0:
                coblk(n, 0, rts[n])
            coblk(n, 1, rts[n], per_chunk_y=(n == n_imgs - 1))


_NC_CACHE = {}


def _get_nc(n_imgs):
    if n_imgs not in _NC_CACHE:
        nc = bacc.Bacc("TRN2", target_bir_lowering=False, debug=False)
        rt_ap = nc.dram_tensor("x", [n_imgs, CI_BLKS, 128, 4, RTL], FP8,
                               kind="ExternalInput").ap()
        wt_ap = nc.dram_tensor("wt", [128, 4, CI_BLKS, 3, COUT], FP8,
                               kind="ExternalInput").ap()
        b_ap = nc.dram_tensor("bias", [COUT], F32, kind="ExternalInput").ap()
        y_ap = nc.dram_tensor("y", [n_imgs, COUT, H * W], FP16,
                              kind="ExternalOutput").ap()
        with tile.TileContext(nc) as tc:
            _build_conv(tc, y_ap, rt_ap, wt_ap, b_ap, n_imgs)
        nc.compile()
        _NC_CACHE[n_imgs] = nc
    return _NC_CACHE[n_imgs]


# fp8e4m3 byte encodings for {-2, -1, 0, +1, +2} indexed by v + 2
# (boundary tile-rows mix pad zeros with +/-1 data, so odd values occur)
_FP8_LUT = np.array([0xC0, 0xB8, 0x00, 0x38, 0x40], dtype=np.uint8)


def make_in_maps(x, weight, bias):
    """Host-side shard prep: sign-binarize x and apply the F(2,3) input
    transform along H (RT_i = d_a +/- d_b over padded stride-2 rows),
    shipping 4 fp8 planes per (image, ci_blk) in the streaming layout
    [n, ci_blk, 128, m, RTL] (data (rtile, j) at col 2 + 58*rtile + j).
    Weights are 1-D Winograd-transformed (G @ g along kh) to fp8."""
    n_imgs = x.shape[0] // N_CORES
    N = x.shape[0]
    xs = np.where(np.asarray(x, dtype=np.float32) >= 0,
                  np.int8(1), np.int8(-1)).reshape(N, CI_BLKS, 128, H, W)
    # padded rows/cols: row r = x row r-1, cols 0..55 data, 56..57 zero
    # (the left-pad of a row is the previous row's col-57 zero in-stream)
    xp = np.zeros((N, CI_BLKS, 128, WP, WP), np.int8)
    xp[:, :, :, 1:57, :W] = xs
    rt = np.zeros((N, CI_BLKS, 128, 4, RTL), np.uint8)
    rtv = rt[:, :, :, :, 2:2 + NRT * WP].reshape(
        N, CI_BLKS, 128, 4, NRT, WP)
    for i, (a, b, s) in enumerate(((0, 2, -1), (1, 2, 1),
                                   (2, 1, -1), (1, 3, -1))):
        v = xp[:, :, :, a:a + 2 * NRT:2] + np.int8(s) * \
            xp[:, :, :, b:b + 2 * NRT:2]
        rtv[:, :, :, i] = _FP8_LUT[v.astype(np.int16) + 2]
    rt8 = rt.view(ml_dtypes.float8_e4m3)

    g = np.where(np.asarray(weight, dtype=np.float32) >= 0,
                 np.float32(1), np.float32(-1))       # [co, ci, kh, kw]
    wt = np.stack([g[:, :, 0, :],
                   (g[:, :, 0, :] + g[:, :, 1, :] + g[:, :, 2, :]) * 0.5,
                   (g[:, :, 0, :] - g[:, :, 1, :] + g[:, :, 2, :]) * 0.5,
                   g[:, :, 2, :]])                    # [4m, co, ci, kw]
    wt = wt.transpose(2, 0, 3, 1).reshape(CI_BLKS, 128, 4, 3, COUT)
    wt8 = np.ascontiguousarray(wt.transpose(1, 2, 0, 3, 4)).astype(
        ml_dtypes.float8_e4m3)                        # [ci_p, m, cib, kw, co]
    b = np.ascontiguousarray(bias, dtype=np.float32)
    return [{"x": np.ascontiguousarray(rt8[i * n_imgs:(i + 1) * n_imgs]),
             "wt": wt8, "bias": b}
            for i in range(N_CORES)]


def kernel(x: np.ndarray, weight: np.ndarray, bias: np.ndarray) -> np.ndarray:
    assert x.shape[1:] == (CIN, H, W), x.shape
    assert x.shape[0] % N_CORES == 0, x.shape
    n_imgs = x.shape[0] // N_CORES
    nc = _get_nc(n_imgs)
    in_maps = make_in_maps(x, weight, bias)
    res = run_bass_kernel_spmd(nc, in_maps, core_ids=list(range(N_CORES)))
    y16 = np.concatenate([r["y"] for r in res.results], axis=0)
    return y16.astype(np.float32).reshape(x.shape[0], COUT, H, W)


# revision 39
# speedup vs baseline: 1.0207x; 1.0207x over previous
"""Binarized 3x3 conv (stride 1, pad 1) + bias on 8 Trainium2 NeuronCores.

Full problem: x[32,256,56,56] f32, weight[256,256,3,3] f32, bias[256] f32
-> y[32,256,56,56] f32 with y = conv2d(sign(x), sign(weight), pad=1) + bias
(sign(t) = +1 for t >= 0 else -1).

Sharding: data-parallel over batch. Each of the 8 cores gets 4 images and a
replicated copy of the weights; the host concatenates the 8 output shards.

Per-core kernel (v15): 1-D Winograd F(2,3) along H, host-side input transform.
  The 3 kh-taps collapse into 4 m-plane matmul groups over stride-2 row
  tiles, cutting streamed PE columns 1.5x vs direct conv (12 matmuls of
  N=406 per 7-tile-row chunk instead of 18 equivalent direct columns).
  Everything stays exact: the host binarizes x to +/-1 and ships the 4
  RT input-transform planes (d_a +/- d_b in {-2..2}) as fp8; the weight
  transform G@g has entries in {+/-0.5, +/-1, +/-1.5} (exact in e4m3);
  products are multiples of 0.5 accumulated in f32 PSUM; y rows are
  m0+m1+m2 / m1-m2-m3 -- half-integer sums well under 2048, so the fp16
  output rounds only the +bias term (~2e-4 rel err).

  - per (image, co_blk, chunk of 7 tile rows): 12 DoubleRow fp8 matmuls
    (4 m-planes x 3 kw-taps) into 4 PSUM banks; the stream is
    LDWEIGHTS-issue-bound at ~171ns/matmul (N=406 cols ~ 169ns);
  - drain/inverse-transform per chunk, split to fit engine budgets
    (PSUM readable only by Scalar/DVE, one operand per op):
      Scalar: s1 = m1 + bias, s3 = m3            (2 ACTs)
      DVE:    t = m0 + s1; ye = t + m2; u = s1 - m2
      GpSimd: yo = u - s3  (last chunk on DVE to shorten the tail)
  - DMA notes: packets split at the innermost AP dim, so transfers are
    emitted partition-major and flattened to one contiguous run per
    partition (6528B rt, 6144B w); rt is split across the sync + gpsimd
    DGE queues by ci_blk; y ships fp16 (halved writeback).
"""

import ml_dtypes
import numpy as np

import concourse.bacc as bacc
import concourse.mybir as mybir
import concourse.tile as tile
from concourse.bass_utils import run_bass_kernel_spmd

F32 = mybir.dt.float32
FP16 = mybir.dt.float16
BF16 = mybir.dt.bfloat16
FP8 = mybir.dt.float8e4
AF = mybir.ActivationFunctionType
ALU = mybir.AluOpType
DR = mybir.MatmulPerfMode.DoubleRow

N_CORES = 8
H = W = 56
WP = 58            # padded row width
CIN = 256
COUT = 256
CI_BLKS = 2
CO_BLKS = 2
NRT = 28           # F(2,3) tile rows (2 output rows each)
RQ = 7             # tile rows per chunk
NQ = NRT // RQ     # 4 chunks per (image, co_blk)
NV = RQ * WP       # 406 matmul moving free size
RTL = 1632         # per-(ci_blk, m) RT plane elems (2 guard + 28*58 + pad)
JUNK_MM = 4


def _build_conv(tc, y_ap, rt_ap, wt_ap, b_ap, n_imgs):
    nc = tc.nc

    with (
        tc.tile_pool(name="consts", bufs=1) as consts,
        tc.tile_pool(name="lhst", bufs=1) as lhst_pool,
        tc.tile_pool(name="rt", bufs=1) as rt_pool,
        tc.tile_pool(name="s1", bufs=3) as s1_pool,
        tc.tile_pool(name="s3", bufs=3) as s3_pool,
        tc.tile_pool(name="tmpe", bufs=2) as tmpe_pool,
        tc.tile_pool(name="tmpo", bufs=2) as tmpo_pool,
        tc.tile_pool(name="outsb", bufs=4) as out_pool,
        tc.tile_pool(name="psum", bufs=8, space="PSUM") as psum_pool,
    ):
        junk = consts.tile([128, 512], BF16, name="junk")
        nc.gpsimd.memset(junk, 0.0)

        # Winograd weights: [128ci_p, 2ci_blk, 4m, 3kw, 256co] fp8.
        # One partition-major transfer: 6144B/partition runs = big DMA
        # packets (per-(m,cb) 768B-run splits crawled at ~38B/ns)
        lhst = lhst_pool.tile([128, CI_BLKS, 4, 3, COUT], FP8)
        nc.scalar.dma_start(
            out=lhst[:, 0:2].rearrange("p a i k o -> p (a i k o)"),
            in_=wt_ap.rearrange("p a i k o -> p (a i k o)"))
        bias_sb = consts.tile([128, CO_BLKS], F32)
        nc.scalar.dma_start(out=bias_sb,
                            in_=b_ap.rearrange("(b p) -> p b", p=128))

        NRTBUF = 3
        rts = [rt_pool.tile([128, CI_BLKS, 4, RTL], FP8, name=f"rt{j}")
               for j in range(NRTBUF)]

        def dma_rt(n, o0, o1):
            # split by ci_blk across two DGE queues (sync + gpsimd): one
            # queue can't sustain the 1.67MB/image against the PE's pace.
            # host layout is partition-major so each transfer streams one
            # contiguous 6528B run per partition
            for cb, eng in ((0, nc.sync), (1, nc.gpsimd)):
                dst = rts[n % NRTBUF][:, cb]
                src = rt_ap[n, cb]
                if o0 == 0 and o1 == RTL:
                    # flatten (m, f): one 6528B run per partition — the DMA
                    # packetizes at the innermost AP dim
                    eng.dma_start(out=dst.rearrange("p i f -> p (i f)"),
                                  in_=src.rearrange("p i f -> p (i f)"))
                else:
                    eng.dma_start(out=dst[:, :, o0:o1],
                                  in_=src[:, :, o0:o1])

        def junk_mm():
            jps = psum_pool.tile([128, 512], F32, name="ps", tag="ps")
            nc.tensor.matmul(jps, junk[:, :128], junk, start=True, stop=True)

        def chunk(n, c, q, rtt, osb, per_chunk_y, tail=False):
            pms = [psum_pool.tile([128, NV], F32, name="ps", tag="ps")
                   for _ in range(4)]
            for i in range(4):
                for kw in range(3):
                    base = WP * RQ * q + kw
                    nc.tensor.matmul(
                        pms[i],
                        lhst[:, 0:2, i, kw, c * 128:(c + 1) * 128],
                        rtt[:, 0:2, i, base:base + NV],
                        start=(kw == 0), stop=(kw == 2), perf_mode=DR)
            pmv = [p.rearrange("p (r w) -> p r w", w=WP)[:, :, 1:57]
                   for p in pms]
            yv = osb.rearrange("p (h2 t w) -> p h2 t w", t=2, w=W)
            ye = yv[:, RQ * q:RQ * (q + 1), 0, :]
            yo = yv[:, RQ * q:RQ * (q + 1), 1, :]
            # PSUM access rules: GpSimd none, DVE/Scalar one operand per op.
            s1 = s1_pool.tile([128, RQ, W], F32, name="s1", tag="s1")
            nc.scalar.activation(out=s1, in_=pmv[1], func=AF.Identity,
                                 bias=bias_sb[:, c:c + 1], scale=1.0)
            s3 = s3_pool.tile([128, RQ, W], F32, name="s3", tag="s3")
            nc.scalar.activation(out=s3, in_=pmv[3], func=AF.Identity)
            tmp_e = tmpe_pool.tile([128, RQ, W], F32, name="te", tag="te")
            nc.vector.tensor_tensor(tmp_e, pmv[0], s1, ALU.add)
            nc.vector.tensor_tensor(ye, tmp_e, pmv[2], ALU.add)
            tmp_o = tmpo_pool.tile([128, RQ, W], F32, name="to", tag="to")
            nc.vector.tensor_tensor(tmp_o, s1, pmv[2], ALU.subtract)
            # final chunk: keep the drain tail off slow GpSimd
            (nc.vector if tail else nc.gpsimd).tensor_tensor(
                yo, tmp_o, s3, ALU.subtract)
            if per_chunk_y:
                dma_y(n, c, osb, 2 * RQ * W * q, 2 * RQ * W * (q + 1))
            elif q == 1:
                dma_y(n, c, osb, 0, H * W // 2)

        def dma_y(n, c, osb, lo, hi):
            nc.scalar.dma_start(
                out=y_ap[n, c * 128:(c + 1) * 128, lo:hi],
                in_=osb[:, lo:hi])

        def coblk(n, c, rtt, per_chunk_y=False):
            osb = out_pool.tile([128, H * W], FP16, name="osb", tag="osb")
            for q in range(NQ):
                chunk(n, c, q, rtt, osb, per_chunk_y,
                      tail=per_chunk_y and q == NQ - 1)
            if not per_chunk_y:
                dma_y(n, c, osb, H * W // 2, H * W)

        # --- image 0 ramp: rt arrives in 4 chunk-aligned pieces, each
        # chunk's matmuls emitted right after its piece so the dep tracker
        # can't chain early chunks onto late pieces; junk matmuls keep the
        # PE clock gate warm meanwhile. Pieces overlap 2 cols (chunk q
        # streams 2 cols into row 7(q+1)'s guard-zero region).
        dma_rt(0, 0, WP * RQ + 2)
        for _ in range(JUNK_MM):
            junk_mm()
        osb0 = out_pool.tile([128, H * W], FP16, name="osb", tag="osb")
        for q in range(NQ):
            chunk(0, 0, q, rts[0], osb0, False)
            if q < NQ - 1:
                dma_rt(0, WP * RQ * (q + 1), min(WP * RQ * (q + 2) + 2, RTL))
        dma_y(0, 0, osb0, H * W // 2, H * W)

        # image n+1's rt DMA (~10us) is issued ~2 sweeps (~16us) ahead
        if n_imgs > 1:
            dma_rt(1, 0, RTL)
        for n in range(n_imgs):
            rtt = rts[n % NRTBUF]
            if n > 0:
                if n + 1 < n_imgs:
                    dma_rt(n + 1, 0, RTL)
                coblk(n, 0, rtt)
            coblk(n, 1, rtt, per_chunk_y=(n == n_imgs - 1))


_NC_CACHE = {}


def _get_nc(n_imgs):
    if n_imgs not in _NC_CACHE:
        nc = bacc.Bacc("TRN2", target_bir_lowering=False, debug=False)
        rt_ap = nc.dram_tensor("x", [n_imgs, CI_BLKS, 128, 4, RTL], FP8,
                               kind="ExternalInput").ap()
        wt_ap = nc.dram_tensor("wt", [128, CI_BLKS, 4, 3, COUT], FP8,
                               kind="ExternalInput").ap()
        b_ap = nc.dram_tensor("bias", [COUT], F32, kind="ExternalInput").ap()
        y_ap = nc.dram_tensor("y", [n_imgs, COUT, H * W], FP16,
                              kind="ExternalOutput").ap()
        with tile.TileContext(nc) as tc:
            _build_conv(tc, y_ap, rt_ap, wt_ap, b_ap, n_imgs)
        nc.compile()
        _NC_CACHE[n_imgs] = nc
    return _NC_CACHE[n_imgs]


# fp8e4m3 byte encodings for {-2, -1, 0, +1, +2} indexed by v + 2
# (boundary tile-rows mix pad zeros with +/-1 data, so odd values occur)
_FP8_LUT = np.array([0xC0, 0xB8, 0x00, 0x38, 0x40], dtype=np.uint8)


def make_in_maps(x, weight, bias):
    """Host-side shard prep: sign-binarize x and apply the F(2,3) input
    transform along H (RT_i = d_a +/- d_b over padded stride-2 rows),
    shipping 4 fp8 planes per (image, ci_blk) in the streaming layout
    [n, ci_blk, 128, m, RTL] (data (rtile, j) at col 2 + 58*rtile + j).
    Weights are 1-D Winograd-transformed (G @ g along kh) to fp8."""
    n_imgs = x.shape[0] // N_CORES
    N = x.shape[0]
    xs = np.where(np.asarray(x, dtype=np.float32) >= 0,
                  np.int8(1), np.int8(-1)).reshape(N, CI_BLKS, 128, H, W)
    # padded rows/cols: row r = x row r-1, cols 0..55 data, 56..57 zero
    # (the left-pad of a row is the previous row's col-57 zero in-stream)
    xp = np.zeros((N, CI_BLKS, 128, WP, WP), np.int8)
    xp[:, :, :, 1:57, :W] = xs
    rt = np.zeros((N, CI_BLKS, 128, 4, RTL), np.uint8)
    rtv = rt[:, :, :, :, 2:2 + NRT * WP].reshape(
        N, CI_BLKS, 128, 4, NRT, WP)
    for i, (a, b, s) in enumerate(((0, 2, -1), (1, 2, 1),
                                   (2, 1, -1), (1, 3, -1))):
        v = xp[:, :, :, a:a + 2 * NRT:2] + np.int8(s) * \
            xp[:, :, :, b:b + 2 * NRT:2]
        rtv[:, :, :, i] = _FP8_LUT[v.astype(np.int16) + 2]
    rt8 = rt.view(ml_dtypes.float8_e4m3)

    g = np.where(np.asarray(weight, dtype=np.float32) >= 0,
                 np.float32(1), np.float32(-1))       # [co, ci, kh, kw]
    wt = np.stack([g[:, :, 0, :],
                   (g[:, :, 0, :] + g[:, :, 1, :] + g[:, :, 2, :]) * 0.5,
                   (g[:, :, 0, :] - g[:, :, 1, :] + g[:, :, 2, :]) * 0.5,
                   g[:, :, 2, :]])                    # [4m, co, ci, kw]
    wt = wt.transpose(2, 0, 3, 1).reshape(CI_BLKS, 128, 4, 3, COUT)
    wt8 = np.ascontiguousarray(wt.transpose(1, 0, 2, 3, 4)).astype(
        ml_dtypes.float8_e4m3)                        # [ci_p, cib, m, kw, co]
    b = np.ascontiguousarray(bias, dtype=np.float32)
    return [{"x": np.ascontiguousarray(rt8[i * n_imgs:(i + 1) * n_imgs]),
             "wt": wt8, "bias": b}
            for i in range(N_CORES)]


def kernel(x: np.ndarray, weight: np.ndarray, bias: np.ndarray) -> np.ndarray:
    assert x.shape[1:] == (CIN, H, W), x.shape
    assert x.shape[0] % N_CORES == 0, x.shape
    n_imgs = x.shape[0] // N_CORES
    nc = _get_nc(n_imgs)
    in_maps = make_in_maps(x, weight, bias)
    res = run_bass_kernel_spmd(nc, in_maps, core_ids=list(range(N_CORES)))
    y16 = np.concatenate([r["y"] for r in res.results], axis=0)
    return y16.astype(np.float32).reshape(x.shape[0], COUT, H, W)
